# revision 1
# baseline (speedup 1.0000x reference)
"""Causal multi-head self-attention (B=4, S=2048, D=1024, H=16) on 8 TRN2
NeuronCores.

Sharding: core c = (batch b = c//2, head-half = c%2). Each core computes, for
its batch and its 8 heads: QKV projections (+RoPE via host-permuted weights
and a signed sin table), causal softmax attention, and a row-sharded output
projection. The host sums the two partial y's per batch.

Device layouts (per core):
  qT,kT: [128, 4, 2048]  chunk hc = heads (2hc, 2hc+1); within a head's 64
         rows: [even dims (32) | odd dims (32)] — RoPE pairs at partition
         offset +32, applied straight out of the projection PSUM.
  v:     [128, 16, 8, 65] = [t % 128, t//128, head, dim+ones]; the 65th
         column of ones makes the attention matmul emit the softmax
         denominator as PSUM row 64.
  scores are built transposed (S.T[t, s]) so exp(S.T) feeds the AV matmul as
  the moving operand with no transposes anywhere: out.T = v.T @ P.T.
  All matmuls run as float32r (fp32 data, reduced-precision multiply at
  1 cycle/row for moving dims >= 256).

Structure notes:
  - PSUM pools are global (tags p1 / sc / pa = 2+4+2 banks): no phase
    boundary PSUM stalls. SBUF pools phase (weights/x close before outT/Wo
    pools open) to fit the 192KB/partition budget.
  - Head-pair-outer attention; each pair's denominators batch into one 8-row
    reciprocal whose DRAM broadcast round-trip trails one head-pair behind.
  - Diagonal score/AV matmuls are column-narrowed to the causal range, which
    also removes any need to zero the masked region of exp tiles.
"""

import numpy as np

B, S, D = 4, 2048, 1024
NUM_HEADS = 16
THETA = 10000.0
DH = 64
N_CORES = 8
P = 128

_CACHE = {}


def build_nc():
    """Build the single-core SPMD Bass program (identical on all 8 cores)."""
    import concourse.mybir as mybir
    import concourse.tile as tile
    from concourse import bacc
    from concourse.bass import ts

    F32 = mybir.dt.float32
    F32R = mybir.dt.float32r
    Act = mybir.ActivationFunctionType

    def r(ap):
        return ap.bitcast(F32R)

    nc = bacc.Bacc(trn_type="TRN2")
    xT_d = nc.dram_tensor("xT", [D, S], F32R, kind="ExternalInput")
    wqT_d = nc.dram_tensor("wqT", [D, 512], F32R, kind="ExternalInput")
    wkT_d = nc.dram_tensor("wkT", [D, 512], F32R, kind="ExternalInput")
    wvT_d = nc.dram_tensor("wvT", [D, 512], F32R, kind="ExternalInput")
    woT_d = nc.dram_tensor("woT", [512, D], F32R, kind="ExternalInput")
    cosT_d = nc.dram_tensor("cosT", [P, S], F32, kind="ExternalInput")
    sinT_d = nc.dram_tensor("sinT", [P, S], F32, kind="ExternalInput")
    tri_d = nc.dram_tensor("tri", [P, P], F32, kind="ExternalInput")
    y_d = nc.dram_tensor("y", [S, D], F32, kind="ExternalOutput")

    xT3 = xT_d.ap().rearrange("(kc p) s -> p kc s", p=P)     # [128, 8, 2048]
    wq3 = wqT_d.ap().rearrange("(kc p) j -> p kc j", p=P)    # [128, 8, 512]
    wk3 = wkT_d.ap().rearrange("(kc p) j -> p kc j", p=P)
    wv3 = wvT_d.ap().rearrange("(kc p) j -> p kc j", p=P)
    wo3 = woT_d.ap().rearrange("(jc p) i -> p jc i", p=P)    # [128, 4, 1024]
    y_ap = y_d.ap()

    with tile.TileContext(nc) as tc:
        with tc.tile_pool(name="pers", bufs=1) as pers:
            qT = pers.tile([P, 4, S], F32R)
            kT = pers.tile([P, 4, S], F32R)
            vA = pers.tile([P, 16, 8, 65], F32R)

            # ---- Phase 1a: q/k projections + RoPE ----
            with (
                tc.tile_pool(name="tab", bufs=1) as tab,
                tc.tile_pool(name="w1", bufs=1) as w1,
                tc.tile_pool(name="x1", bufs=2) as x1,
                tc.tile_pool(name="tmp1", bufs=2) as tmp1,
                tc.tile_pool(name="ps1", bufs=4, space="PSUM") as psA,
            ):
                wq_s = w1.tile([P, 8, 512], F32R)
                wk_s = w1.tile([P, 8, 512], F32R)
                cosb = tab.tile([P, S], F32)
                sinb = tab.tile([P, S], F32)
                for kc in range(8):  # chunked so kc=0 arrives early
                    nc.sync.dma_start(wq_s[:, kc, :], wq3[:, kc, :])
                    nc.sync.dma_start(wk_s[:, kc, :], wk3[:, kc, :])
                nc.sync.dma_start(cosb[:], cosT_d.ap())
                nc.sync.dma_start(sinb[:], sinT_d.ap())

                def rope(pq, dst2d, sls):
                    # tA = proj * cos (full width); tBs = 32-row-swapped proj
                    # times the SIGNED sin table (+sin top rows, -sin bottom
                    # rows); combine with one full-width add: r = tA + tBs
                    tA = tmp1.tile([P, 512], F32, tag="tA")
                    nc.vector.tensor_mul(tA[:], pq[:], cosb[:, sls])
                    tBs = tmp1.tile([P, 512], F32, tag="tBs")
                    for hb in (0, 64):
                        nc.vector.tensor_mul(
                            tBs[hb : hb + 32, :],
                            pq[hb + 32 : hb + 64, :], sinb[hb + 32 : hb + 64, sls],
                        )
                        nc.vector.tensor_mul(
                            tBs[hb + 32 : hb + 64, :],
                            pq[hb : hb + 32, :], sinb[hb : hb + 32, sls],
                        )
                    nc.vector.tensor_add(dst2d, tA[:], tBs[:])

                for sl in range(4):
                    sls = ts(sl, 512)
                    xs = x1.tile([P, 8, 512], F32R, tag="xs")
                    for kc in range(8):
                        nc.sync.dma_start(xs[:, kc, :], xT3[:, kc, sls])
                    for jc in range(4):
                        pq = psA.tile([P, 512], F32, tag="p1")
                        for kc in range(8):
                            nc.tensor.matmul(
                                pq[:], r(wq_s[:, kc, ts(jc, P)]), r(xs[:, kc, :]),
                                start=(kc == 0), stop=(kc == 7),
                            )
                        rope(pq, qT[:, jc, sls], sls)
                        pk = psA.tile([P, 512], F32, tag="p1")
                        for kc in range(8):
                            nc.tensor.matmul(
                                pk[:], r(wk_s[:, kc, ts(jc, P)]), r(xs[:, kc, :]),
                                start=(kc == 0), stop=(kc == 7),
                            )
                        rope(pk, kT[:, jc, sls], sls)

            # ---- Phase 1b: v projection ----
            with (
                tc.tile_pool(name="w2", bufs=1) as w2,
                tc.tile_pool(name="x2", bufs=2) as x2,
                tc.tile_pool(name="ps2", bufs=4, space="PSUM") as psA,
            ):
                wv_s = w2.tile([P, 8, 512], F32R)
                nc.sync.dma_start(wv_s[:], wv3)
                # f32r memset isn't a legal ISA value type; write the ones
                # columns via tensor_copy from a small fp32 tile instead
                ones8 = w2.tile([P, 8], F32)
                nc.vector.memset(ones8[:], 1.0)
                for t16 in range(16):
                    nc.vector.tensor_copy(vA[:, t16, :, 64:65], ones8.unsqueeze(2))
                for sl in range(4):
                    xs2 = x2.tile([P, 8, 512], F32R, tag="xs2")
                    nc.sync.dma_start(xs2[:], xT3[:, :, ts(sl, 512)])
                    for t4i in range(4):
                        pv = psA.tile([P, 512], F32, tag="p1")
                        for kc in range(8):
                            nc.tensor.matmul(
                                pv[:], r(xs2[:, kc, ts(t4i, P)]), r(wv_s[:, kc, :]),
                                start=(kc == 0), stop=(kc == 7),
                            )
                        nc.vector.tensor_copy(
                            vA[:, sl * 4 + t4i, :, 0:64],
                            pv.rearrange("p (h c) -> p h c", h=8),
                        )

            # ---- Phase 2: attention, head-pair outer ----
            with (
                tc.tile_pool(name="wo", bufs=1) as wo,
                tc.tile_pool(name="outp", bufs=1) as outp,
                tc.tile_pool(name="trip", bufs=1) as trip,
                tc.tile_pool(name="ptp", bufs=4) as ptp,
                tc.tile_pool(name="rcp", bufs=3) as rcp,
                tc.tile_pool(name="rbp", bufs=4) as rbp,
                tc.tile_pool(name="ysb", bufs=2) as ysb,
                tc.tile_pool(name="drm", bufs=2, space="DRAM") as drm,
            ):
                _psB_cm = tc.tile_pool(name="psB", bufs=2, space="PSUM")
                _psC_cm = tc.tile_pool(name="psC", bufs=2, space="PSUM")
                psB = _psB_cm.__enter__()
                psC = _psC_cm.__enter__()
                wo_s = wo.tile([P, 4, D], F32R)
                nc.sync.dma_start(wo_s[:], wo3)
                outT = outp.tile([P, 4, S], F32R)
                trib = trip.tile([P, P], F32)
                nc.sync.dma_start(trib[:], tri_d.ap())

                den_tiles = {}

                def attention_pair(hc):
                    den_d = drm.tile([8, 512], F32, tag="dend")
                    den_tiles[hc] = den_d
                    for j in range(4):
                        # both heads' scores/exp/AV share paired [*, 1024]
                        # tiles: head0 in cols 0:512, head1 in 512:1024
                        pa = psC.tile([65, 1024], F32, tag="pa")
                        last = 4 * j + 3
                        for i in range(last + 1):
                            m = i - 4 * j
                            w0 = max(m, 0) * P   # first causal col in the 512
                            sc = psB.tile([P, 1024], F32, tag="sc")
                            nc.tensor.matmul(
                                sc[:, w0:512], r(kT[0:64, hc, ts(i, P)]),
                                r(qT[0:64, hc, j * 512 + w0 : (j + 1) * 512]),
                                start=True, stop=True,
                            )
                            nc.tensor.matmul(
                                sc[:, 512 + w0 : 1024], r(kT[64:P, hc, ts(i, P)]),
                                r(qT[64:P, hc, j * 512 + w0 : (j + 1) * 512]),
                                start=True, stop=True,
                            )
                            pt = ptp.tile([P, 1024], F32R, tag="pt")
                            if m < 0:
                                nc.scalar.activation(pt[:], sc[:], Act.Exp)
                            else:
                                nc.scalar.activation(
                                    pt[:, w0:512], sc[:, w0:512], Act.Exp
                                )
                                nc.scalar.activation(
                                    pt[:, 512 + w0 : 1024],
                                    sc[:, 512 + w0 : 1024], Act.Exp,
                                )
                                nc.vector.tensor_mul(
                                    pt[:, w0 : w0 + P], pt[:, w0 : w0 + P], trib[:]
                                )
                                nc.vector.tensor_mul(
                                    pt[:, 512 + w0 : 512 + w0 + P],
                                    pt[:, 512 + w0 : 512 + w0 + P], trib[:],
                                )
                            nc.tensor.matmul(
                                pa[:, w0:512], r(vA[:, i, 2 * hc, :]),
                                r(pt[:, w0:512]),
                                start=(i == 0), stop=(i == last),
                            )
                            nc.tensor.matmul(
                                pa[:, 512 + w0 : 1024], r(vA[:, i, 2 * hc + 1, :]),
                                r(pt[:, 512 + w0 : 1024]),
                                start=(i == 0), stop=(i == last),
                            )
                        # release pa quickly: unnormalized out rows and
                        # denominator rows (both DVE; ScalarE paces the exp)
                        for h01 in range(2):
                            hb = h01 * 64
                            cs0 = h01 * 512
                            nc.vector.tensor_copy(
                                outT[hb : hb + 64, hc, ts(j, 512)],
                                pa[0:64, cs0 : cs0 + 512],
                            )
                            srow = rcp.tile([1, 512], F32, tag="srow")
                            nc.vector.tensor_copy(srow[:], pa[64:65, cs0 : cs0 + 512])
                            nc.sync.dma_start(
                                den_d[j * 2 + h01 : j * 2 + h01 + 1, :], srow[:]
                            )

                def epilogue_pair(hc, p3=None):
                    # batched denominators: one 8-row reciprocal, broadcast
                    # rows back through DRAM, divide in place. When p3 is
                    # set (last pair), interleave each j-block's divisions
                    # with that block's output-projection tiles.
                    den_d = den_tiles[hc]
                    den_sb = rcp.tile([8, 512], F32, tag="densb")
                    nc.sync.dma_start(den_sb[:], den_d[:])
                    rec8 = rcp.tile([8, 512], F32, tag="rec8")
                    nc.vector.reciprocal(rec8[:], den_sb[:])
                    rec_d = drm.tile([8, 512], F32, tag="recd")
                    nc.sync.dma_start(rec_d[:], rec8[:])
                    for j in range(4):
                        for h01 in range(2):
                            rb = rbp.tile([P, 512], F32, tag="rb")
                            row = j * 2 + h01
                            hb = h01 * 64
                            nc.sync.dma_start(
                                rb[hb : hb + 64, :],
                                rec_d[row : row + 1, :].broadcast_to((64, 512)),
                            )
                            nc.vector.tensor_mul(
                                outT[hb : hb + 64, hc, ts(j, 512)],
                                outT[hb : hb + 64, hc, ts(j, 512)],
                                rb[hb : hb + 64, :],
                            )
                        if p3 is not None:
                            p3(j)

                # epilogues trail one head-pair behind so their DMA round-trip
                # latency hides under the next pair's dense compute; the last
                # pair's divisions interleave with the output projection
                attention_pair(0)
                for hc in range(1, 4):
                    attention_pair(hc)
                    epilogue_pair(hc - 1)
                _psC_cm.__exit__(None, None, None)
                _psB_cm.__exit__(None, None, None)

                # ---- Phase 3: output projection y = outT.T @ woT ----
                ps3 = tc.tile_pool(name="ps3", bufs=2, space="PSUM")
                ps3p = ps3.__enter__()

                def p3_group(j):
                    for st in range(4 * j, 4 * j + 4):
                        py0 = ps3p.tile([P, 512], F32, tag="py0")
                        py1 = ps3p.tile([P, 512], F32, tag="py1")
                        for jc in range(4):
                            nc.tensor.matmul(
                                py0[:], r(outT[:, jc, ts(st, P)]),
                                r(wo_s[:, jc, 0:512]),
                                start=(jc == 0), stop=(jc == 3),
                            )
                        for jc in range(4):
                            nc.tensor.matmul(
                                py1[:], r(outT[:, jc, ts(st, P)]),
                                r(wo_s[:, jc, 512:D]),
                                start=(jc == 0), stop=(jc == 3),
                            )
                        yo0 = ysb.tile([P, 512], F32, tag="yo0")
                        yo1 = ysb.tile([P, 512], F32, tag="yo1")
                        nc.scalar.copy(yo0[:], py0[:])
                        nc.scalar.copy(yo1[:], py1[:])
                        nc.sync.dma_start(y_ap[ts(st, P), 0:512], yo0[:])
                        nc.sync.dma_start(y_ap[ts(st, P), 512:D], yo1[:])

                epilogue_pair(3)
                for _j in range(4):
                    p3_group(_j)
                ps3.__exit__(None, None, None)

    nc.compile()
    return nc


def prep_core_inputs(x, token_ids, Wq, Wk, Wv, Wo, core):
    b, half = divmod(core, 2)
    rows = []
    for h in range(half * 8, half * 8 + 8):
        base = h * DH
        rows.extend(base + np.arange(0, DH, 2))
        rows.extend(base + np.arange(1, DH, 2))
    rows = np.asarray(rows)
    cols = np.arange(half * 512, half * 512 + 512)

    f32 = np.float32
    inv = THETA ** (-np.arange(0, DH, 2, dtype=np.float64) / DH)
    ang = np.asarray(token_ids, dtype=np.float64)[None, :] * inv[:, None]
    cosT = np.tile(np.cos(ang), (4, 1)).astype(f32)
    sin_block = np.concatenate([np.sin(ang), -np.sin(ang)], axis=0)
    sinT = np.tile(sin_block, (2, 1)).astype(f32)
    tri = (np.arange(P)[:, None] <= np.arange(P)[None, :]).astype(f32)
    return {
        "xT": np.ascontiguousarray(np.asarray(x[b], f32).T),
        "wqT": np.ascontiguousarray((np.asarray(Wq, f32)[rows] * 0.125).T),
        "wkT": np.ascontiguousarray(np.asarray(Wk, f32)[rows].T),
        "wvT": np.ascontiguousarray(np.asarray(Wv, f32)[cols].T),
        "woT": np.ascontiguousarray(np.asarray(Wo, f32)[:, cols].T),
        "cosT": cosT,
        "sinT": sinT,
        "tri": tri,
    }


def get_nc():
    if "nc" not in _CACHE:
        _CACHE["nc"] = build_nc()
    return _CACHE["nc"]


def run_cores(in_maps, trace=False):
    from concourse.bass_utils import run_bass_kernel_spmd

    return run_bass_kernel_spmd(
        get_nc(), in_maps, core_ids=list(range(N_CORES)), trace=trace
    )


def kernel(x, token_ids, Wq, Wk, Wv, Wo):
    in_maps = [
        prep_core_inputs(x, token_ids, Wq, Wk, Wv, Wo, c) for c in range(N_CORES)
    ]
    res = run_cores(in_maps)
    y = np.empty((B, S, D), np.float32)
    for b in range(B):
        y[b] = res.results[2 * b]["y"] + res.results[2 * b + 1]["y"]
    return y



# revision 13
# speedup vs baseline: 1.1337x; 1.1337x over previous
"""Causal multi-head self-attention (B=4, S=2048, D=1024, H=16) on 8 TRN2
NeuronCores.

Sharding: core c = (batch b = c//2, head-half = c%2). Each core computes, for
its batch and its 8 heads: fused QKV projections (+RoPE), causal softmax
attention, and a row-sharded output projection; the host sums the two partial
y's per batch.

v2 (vs the fp32r baseline): all matmul operands are bf16 (halves DMA, removes
the fp32r narrow-moving 4x penalty, enables DVE 2x modes); q/k/v projections
share one pass over x (no phase boundary); RoPE's partition swap runs as 4
small SBUF->SBUF DMAs off the compute engines (ScalarE downcasts the PSUM
projection to bf16, DVE does 2 muls + 1 add at 2x) instead of 4 full-price
partition-sliced DVE muls; attention is query-block-outer so the output
projection of block j interleaves into block j+1's PE stream; per-head [65,512]
PSUM accumulators (+ ones column emitting softmax denominators) keep all 8
PSUM banks allocated: 2x2 scores + 3 pa + 1 output-projection.

Device layouts (per core):
  qT,kT: [128, 4, 2048] bf16; chunk hc = heads (2hc, 2hc+1); within a head's 64
         rows: [even dims (32) | odd dims (32)].
  v:     [128, 16, 8, 65] bf16 = [t % 128, t//128, head, dim+ones]; the ones
         column makes the AV matmul emit the softmax denominator as row 64.
  scores are built transposed (S.T[t, s]) so exp(S.T) feeds the AV matmul as
  the moving operand with no transposes anywhere.
"""

import numpy as np

B, S, D = 4, 2048, 1024
NUM_HEADS = 16
THETA = 10000.0
DH = 64
N_CORES = 8
P = 128

_CACHE = {}


def build_nc():
    """Build the single-core SPMD Bass program (identical on all 8 cores)."""
    import concourse.mybir as mybir
    import concourse.tile as tile
    from concourse import bacc
    from concourse.bass import ts

    F32 = mybir.dt.float32
    BF16 = mybir.dt.bfloat16
    Act = mybir.ActivationFunctionType

    nc = bacc.Bacc(trn_type="TRN2")
    xT_d = nc.dram_tensor("xT", [D, S], BF16, kind="ExternalInput")
    wqT_d = nc.dram_tensor("wqT", [D, 512], BF16, kind="ExternalInput")
    wkT_d = nc.dram_tensor("wkT", [D, 512], BF16, kind="ExternalInput")
    wvT_d = nc.dram_tensor("wvT", [D, 512], BF16, kind="ExternalInput")
    woT_d = nc.dram_tensor("woT", [512, D], BF16, kind="ExternalInput")
    cosT_d = nc.dram_tensor("cosT", [P, S], BF16, kind="ExternalInput")
    sinT_d = nc.dram_tensor("sinT", [P, S], BF16, kind="ExternalInput")
    tri_d = nc.dram_tensor("tri", [P, P], BF16, kind="ExternalInput")
    y_d = nc.dram_tensor("y", [S, D], F32, kind="ExternalOutput")

    xT3 = xT_d.ap().rearrange("(kc p) s -> p kc s", p=P)     # [128, 8, 2048]
    wq3 = wqT_d.ap().rearrange("(kc p) j -> p kc j", p=P)    # [128, 8, 512]
    wk3 = wkT_d.ap().rearrange("(kc p) j -> p kc j", p=P)
    wv3 = wvT_d.ap().rearrange("(kc p) j -> p kc j", p=P)
    wo3 = woT_d.ap().rearrange("(jc p) i -> p jc i", p=P)    # [128, 4, 1024]
    y_ap = y_d.ap()

    with tile.TileContext(nc) as tc:
        with tc.tile_pool(name="pers", bufs=1) as pers:
            qT = pers.tile([P, 4, S], BF16)
            kT = pers.tile([P, 4, S], BF16)
            vA = pers.tile([P, 16, 8, 65], BF16)
            outT = pers.tile([P, 4, S], BF16)
            wo_s = pers.tile([P, 4, D], BF16)
            trib = pers.tile([P, P], BF16)
            cosb = pers.tile([P, S], BF16)
            sinb = pers.tile([P, S], BF16)

            # ---- Phase 1: fused q/k/v projections + RoPE ----
            with (
                tc.tile_pool(name="w1", bufs=1) as w1,
                tc.tile_pool(name="x1", bufs=2) as x1,
                tc.tile_pool(name="rt", bufs=3) as rt,
                tc.tile_pool(name="ps1", bufs=4, space="PSUM") as ps1,
            ):
                wq_s = w1.tile([P, 8, 512], BF16)
                wk_s = w1.tile([P, 8, 512], BF16)
                wv_s = w1.tile([P, 8, 512], BF16)
                ones8 = w1.tile([P, 8], F32)

                # DMA priority order: first q-projection inputs, then rope
                # tables / k / v weights, then the phase-2/3 constants.
                xs0 = x1.tile([P, 8, 512], BF16, tag="xs")
                for kc in range(8):
                    nc.sync.dma_start(xs0[:, kc, :], xT3[:, kc, 0:512])
                    nc.sync.dma_start(wq_s[:, kc, :], wq3[:, kc, :])
                nc.sync.dma_start(cosb[:], cosT_d.ap())
                nc.sync.dma_start(sinb[:], sinT_d.ap())
                for kc in range(8):
                    nc.sync.dma_start(wk_s[:, kc, :], wk3[:, kc, :])
                nc.sync.dma_start(wv_s[:], wv3)
                nc.sync.dma_start(wo_s[:], wo3)
                nc.sync.dma_start(trib[:], tri_d.ap())

                nc.vector.memset(ones8[:], 1.0)
                for t16 in range(16):
                    nc.vector.tensor_copy(vA[:, t16, :, 64:65], ones8.unsqueeze(2))

                def rope(pq, dst2d, sls):
                    # pqb = bf16(pq) on ScalarE (pays the PSUM read once);
                    # then all-bf16 SBUF muls run at DVE 2x: tA = pqb*cos,
                    # tB = swap32(pqb)*sin with the sign folded into the
                    # (destination-row) sin table; r = tA + tB.
                    pqb = rt.tile([P, 512], BF16, tag="pqb")
                    nc.scalar.activation(pqb[:], pq[:], Act.Copy)
                    tA = rt.tile([P, 512], BF16, tag="tA")
                    nc.vector.tensor_mul(tA[:], pqb[:], cosb[:, sls])
                    tB = rt.tile([P, 512], BF16, tag="tB")
                    for hb in (0, 64):
                        nc.vector.tensor_mul(
                            tB[hb : hb + 32, :],
                            pqb[hb + 32 : hb + 64, :], sinb[hb + 32 : hb + 64, sls],
                        )
                        nc.vector.tensor_mul(
                            tB[hb + 32 : hb + 64, :],
                            pqb[hb : hb + 32, :], sinb[hb : hb + 32, sls],
                        )
                    nc.vector.tensor_add(dst2d, tA[:], tB[:])

                for sl in range(4):
                    sls = ts(sl, 512)
                    if sl == 0:
                        xs = xs0
                    else:
                        xs = x1.tile([P, 8, 512], BF16, tag="xs")
                        for kc in range(8):
                            nc.sync.dma_start(xs[:, kc, :], xT3[:, kc, sls])
                    for jc in range(4):
                        pq = ps1.tile([P, 512], F32, tag="p1")
                        for kc in range(8):
                            nc.tensor.matmul(
                                pq[:], wq_s[:, kc, ts(jc, P)], xs[:, kc, :],
                                start=(kc == 0), stop=(kc == 7),
                            )
                        rope(pq, qT[:, jc, sls], sls)
                        pk = ps1.tile([P, 512], F32, tag="p1")
                        for kc in range(8):
                            nc.tensor.matmul(
                                pk[:], wk_s[:, kc, ts(jc, P)], xs[:, kc, :],
                                start=(kc == 0), stop=(kc == 7),
                            )
                        rope(pk, kT[:, jc, sls], sls)
                    for t4i in range(4):
                        pv = ps1.tile([P, 512], F32, tag="p1")
                        for kc in range(8):
                            nc.tensor.matmul(
                                pv[:], xs[:, kc, ts(t4i, P)], wv_s[:, kc, :],
                                start=(kc == 0), stop=(kc == 7),
                            )
                        nc.vector.tensor_copy(
                            vA[:, sl * 4 + t4i, :, 0:64],
                            pv.rearrange("p (h c) -> p h c", h=8),
                        )

            # ---- Phase 2: attention (query-block outer) + interleaved
            # ---- phase 3 (output projection y = outT.T @ woT) ----
            with (
                tc.tile_pool(name="ptp", bufs=4) as ptp,
                tc.tile_pool(name="rcp", bufs=4) as rcp,
                tc.tile_pool(name="rbp", bufs=4) as rbp,
                tc.tile_pool(name="ysb", bufs=3) as ysb,
                tc.tile_pool(name="drm", bufs=2, space="DRAM") as drm,
                tc.tile_pool(name="psB", bufs=2, space="PSUM") as psB,
                tc.tile_pool(name="psC", bufs=3, space="PSUM") as psC,
                tc.tile_pool(name="ps3", bufs=1, space="PSUM") as ps3,
            ):

                def attention_block(j, hc):
                    pa0 = psC.tile([65, 512], F32, tag="pa")
                    pa1 = psC.tile([65, 512], F32, tag="pa")
                    last = 4 * j + 3
                    for i in range(last + 1):
                        m = i - 4 * j
                        w0 = max(m, 0) * P
                        sc = psB.tile([P, 1024], F32, tag="sc")
                        nc.tensor.matmul(
                            sc[:, w0:512], kT[0:64, hc, ts(i, P)],
                            qT[0:64, hc, j * 512 + w0 : (j + 1) * 512],
                            start=True, stop=True,
                        )
                        nc.tensor.matmul(
                            sc[:, 512 + w0 : 1024], kT[64:P, hc, ts(i, P)],
                            qT[64:P, hc, j * 512 + w0 : (j + 1) * 512],
                            start=True, stop=True,
                        )
                        pt = ptp.tile([P, 1024], BF16, tag="pt")
                        if m < 0:
                            nc.scalar.activation(pt[:], sc[:], Act.Exp)
                        else:
                            nc.scalar.activation(
                                pt[:, w0:512], sc[:, w0:512], Act.Exp
                            )
                            nc.scalar.activation(
                                pt[:, 512 + w0 : 1024],
                                sc[:, 512 + w0 : 1024], Act.Exp,
                            )
                            nc.vector.tensor_mul(
                                pt[:, w0 : w0 + P], pt[:, w0 : w0 + P], trib[:]
                            )
                            nc.vector.tensor_mul(
                                pt[:, 512 + w0 : 512 + w0 + P],
                                pt[:, 512 + w0 : 512 + w0 + P], trib[:],
                            )
                        nc.tensor.matmul(
                            pa0[:, w0:512], vA[:, i, 2 * hc, :], pt[:, w0:512],
                            start=(i == 0), stop=(i == last),
                        )
                        nc.tensor.matmul(
                            pa1[:, w0:512], vA[:, i, 2 * hc + 1, :],
                            pt[:, 512 + w0 : 1024],
                            start=(i == 0), stop=(i == last),
                        )
                    rec_d = rec_tiles[j]
                    for h01, pa in ((0, pa0), (1, pa1)):
                        hb = h01 * 64
                        nc.vector.tensor_copy(
                            outT[hb : hb + 64, hc, ts(j, 512)], pa[0:64, :]
                        )
                        r = 2 * hc + h01
                        rc = rcp.tile([1, 512], BF16, tag="rc")
                        with nc.allow_low_precision(reason="bf16 softmax normalizer"):
                            nc.vector.reciprocal(rc[:], pa[64:65, :])
                        nc.sync.dma_start(rec_d[r : r + 1, :], rc[:])

                def norm_block(j):
                    # broadcast the reciprocal denominator rows from DRAM and
                    # multiply outT's j-block in place
                    rec_d = rec_tiles[j]
                    for hc in range(4):
                        for h01 in range(2):
                            r = 2 * hc + h01
                            hb = h01 * 64
                            rb = rbp.tile([P, 512], BF16, tag="rb")
                            nc.sync.dma_start(
                                rb[hb : hb + 64, :],
                                rec_d[r : r + 1, :].broadcast_to((64, 512)),
                            )
                            nc.vector.tensor_mul(
                                outT[hb : hb + 64, hc, ts(j, 512)],
                                outT[hb : hb + 64, hc, ts(j, 512)],
                                rb[hb : hb + 64, :],
                            )

                def p3_group(j, sts):
                    for st in sts:
                        for half in range(2):
                            py = ps3.tile([P, 512], F32, tag="py")
                            for jc in range(4):
                                nc.tensor.matmul(
                                    py[:], outT[:, jc, ts(st, P)],
                                    wo_s[:, jc, half * 512 : (half + 1) * 512],
                                    start=(jc == 0), stop=(jc == 3),
                                )
                            yo = ysb.tile([P, 512], F32, tag="yo")
                            nc.vector.tensor_copy(yo[:], py[:])
                            nc.sync.dma_start(
                                y_ap[ts(st, P), half * 512 : (half + 1) * 512],
                                yo[:],
                            )

                rec_tiles = {}
                for j in range(4):
                    rec_tiles[j] = drm.tile([8, 512], BF16, tag="recd", name="recd")
                    for hc in range(4):
                        attention_block(j, hc)
                        # p3 of the previous block interleaves into this
                        # block's PE stream, two query-chunks at a time,
                        # starting late enough that norm(j-1) has resolved
                        if j >= 1 and hc >= 2:
                            p3_group(j - 1, [4 * (j - 1) + 2 * (hc - 2) + k for k in range(2)])
                    norm_block(j)
                p3_group(3, [12, 13, 14, 15])

    nc.compile()
    return nc


def prep_core_inputs(x, token_ids, Wq, Wk, Wv, Wo, core):
    import ml_dtypes

    bf16 = ml_dtypes.bfloat16
    b, half = divmod(core, 2)
    rows = []
    for h in range(half * 8, half * 8 + 8):
        base = h * DH
        rows.extend(base + np.arange(0, DH, 2))
        rows.extend(base + np.arange(1, DH, 2))
    rows = np.asarray(rows)
    cols = np.arange(half * 512, half * 512 + 512)

    f32 = np.float32
    inv = THETA ** (-np.arange(0, DH, 2, dtype=np.float64) / DH)
    ang = np.asarray(token_ids, dtype=np.float64)[None, :] * inv[:, None]
    cosT = np.tile(np.cos(ang), (4, 1)).astype(bf16)
    # sign folded per SOURCE row (verifier wants both mul inputs on the same
    # partitions): odd-dim source rows carry -sin (r1 = x1 c - x2 s), even-dim
    # source rows carry +sin (r2 = x2 c + x1 s)
    sin_block = np.concatenate([np.sin(ang), -np.sin(ang)], axis=0)
    sinT = np.tile(sin_block, (2, 1)).astype(bf16)
    tri = (np.arange(P)[:, None] <= np.arange(P)[None, :]).astype(bf16)
    return {
        "xT": np.ascontiguousarray(np.asarray(x, f32)[b].T).astype(bf16),
        "wqT": np.ascontiguousarray((np.asarray(Wq, f32)[rows] * 0.125).T).astype(bf16),
        "wkT": np.ascontiguousarray(np.asarray(Wk, f32)[rows].T).astype(bf16),
        "wvT": np.ascontiguousarray(np.asarray(Wv, f32)[cols].T).astype(bf16),
        "woT": np.ascontiguousarray(np.asarray(Wo, f32)[:, cols].T).astype(bf16),
        "cosT": cosT,
        "sinT": sinT,
        "tri": tri,
    }


def get_nc():
    if "nc" not in _CACHE:
        _CACHE["nc"] = build_nc()
    return _CACHE["nc"]


def run_cores(in_maps, trace=False):
    from concourse.bass_utils import run_bass_kernel_spmd

    return run_bass_kernel_spmd(
        get_nc(), in_maps, core_ids=list(range(N_CORES)), trace=trace
    )


def kernel(x, token_ids, Wq, Wk, Wv, Wo):
    in_maps = [
        prep_core_inputs(x, token_ids, Wq, Wk, Wv, Wo, c) for c in range(N_CORES)
    ]
    res = run_cores(in_maps)
    y = np.empty((B, S, D), np.float32)
    for b in range(B):
        y[b] = res.results[2 * b]["y"] + res.results[2 * b + 1]["y"]
    return y


# revision 17
# speedup vs baseline: 1.2681x; 1.1185x over previous
"""Causal multi-head self-attention (B=4, S=2048, D=1024, H=16) on 8 TRN2
NeuronCores.

Sharding: core c = (batch b = c//2, head-half = c%2). Each core computes, for
its batch and its 8 heads: fused QKV projections (+RoPE), causal softmax
attention, and a row-sharded output projection; the host sums the two partial
y's per batch.

v2 (vs the fp32r baseline): all matmul operands are bf16 (halves DMA, removes
the fp32r narrow-moving 4x penalty, enables DVE 2x modes); q/k/v projections
share one pass over x (no phase boundary); RoPE's partition swap runs as 4
small SBUF->SBUF DMAs off the compute engines (ScalarE downcasts the PSUM
projection to bf16, DVE does 2 muls + 1 add at 2x) instead of 4 full-price
partition-sliced DVE muls; attention is query-block-outer so the output
projection of block j interleaves into block j+1's PE stream; per-head [65,512]
PSUM accumulators (+ ones column emitting softmax denominators) keep all 8
PSUM banks allocated: 2x2 scores + 3 pa + 1 output-projection.

Device layouts (per core):
  qT,kT: [128, 4, 2048] bf16; chunk hc = heads (2hc, 2hc+1); within a head's 64
         rows: [even dims (32) | odd dims (32)].
  v:     [128, 16, 8, 65] bf16 = [t % 128, t//128, head, dim+ones]; the ones
         column makes the AV matmul emit the softmax denominator as row 64.
  scores are built transposed (S.T[t, s]) so exp(S.T) feeds the AV matmul as
  the moving operand with no transposes anywhere.
"""

import numpy as np

B, S, D = 4, 2048, 1024
NUM_HEADS = 16
THETA = 10000.0
DH = 64
N_CORES = 8
P = 128

_CACHE = {}


def build_nc():
    """Build the single-core SPMD Bass program (identical on all 8 cores)."""
    import concourse.mybir as mybir
    import concourse.tile as tile
    from concourse import bacc
    from concourse.bass import ts

    F32 = mybir.dt.float32
    BF16 = mybir.dt.bfloat16
    Act = mybir.ActivationFunctionType

    nc = bacc.Bacc(trn_type="TRN2")
    xT_d = nc.dram_tensor("xT", [D, S], BF16, kind="ExternalInput")
    wqT_d = nc.dram_tensor("wqT", [D, 512], BF16, kind="ExternalInput")
    wkT_d = nc.dram_tensor("wkT", [D, 512], BF16, kind="ExternalInput")
    wvT_d = nc.dram_tensor("wvT", [D, 512], BF16, kind="ExternalInput")
    woT_d = nc.dram_tensor("woT", [512, D], BF16, kind="ExternalInput")
    cosT_d = nc.dram_tensor("cosT", [P, S], BF16, kind="ExternalInput")
    sinT_d = nc.dram_tensor("sinT", [P, S], BF16, kind="ExternalInput")
    tri_d = nc.dram_tensor("tri", [P, P], BF16, kind="ExternalInput")
    y_d = nc.dram_tensor("y", [S, D], F32, kind="ExternalOutput")

    xT3 = xT_d.ap().rearrange("(kc p) s -> p kc s", p=P)     # [128, 8, 2048]
    wq3 = wqT_d.ap().rearrange("(kc p) j -> p kc j", p=P)    # [128, 8, 512]
    wk3 = wkT_d.ap().rearrange("(kc p) j -> p kc j", p=P)
    wv3 = wvT_d.ap().rearrange("(kc p) j -> p kc j", p=P)
    wo3 = woT_d.ap().rearrange("(jc p) i -> p jc i", p=P)    # [128, 4, 1024]
    y_ap = y_d.ap()

    with tile.TileContext(nc) as tc:
        with tc.tile_pool(name="pers", bufs=1) as pers:
            qT = pers.tile([P, 4, S], BF16)
            kT = pers.tile([P, 4, S], BF16)
            vA = pers.tile([P, 16, 8, 65], BF16)
            outT = pers.tile([P, 4, S], BF16)
            wo_s = pers.tile([P, 4, D], BF16)
            trib = pers.tile([P, P], BF16)
            cosb = pers.tile([P, S], BF16)
            sinb = pers.tile([P, S], BF16)

            # ---- Phase 1: fused q/k/v projections + RoPE ----
            with (
                tc.tile_pool(name="w1", bufs=1) as w1,
                tc.tile_pool(name="x1", bufs=2) as x1,
                tc.tile_pool(name="rt", bufs=3) as rt,
                tc.tile_pool(name="ps1", bufs=4, space="PSUM") as ps1,
            ):
                wq_s = w1.tile([P, 8, 512], BF16)
                wk_s = w1.tile([P, 8, 512], BF16)
                wv_s = w1.tile([P, 8, 512], BF16)
                ones8 = w1.tile([P, 8], F32)

                # DMA priority order: first q-projection inputs, then rope
                # tables / k / v weights, then the phase-2/3 constants.
                xs0 = x1.tile([P, 8, 512], BF16, tag="xs")
                for kc in range(8):
                    nc.sync.dma_start(xs0[:, kc, :], xT3[:, kc, 0:512])
                    nc.sync.dma_start(wq_s[:, kc, :], wq3[:, kc, :])
                nc.sync.dma_start(cosb[:], cosT_d.ap())
                nc.sync.dma_start(sinb[:], sinT_d.ap())
                for kc in range(8):
                    nc.sync.dma_start(wk_s[:, kc, :], wk3[:, kc, :])
                nc.sync.dma_start(wv_s[:], wv3)
                nc.sync.dma_start(wo_s[:], wo3)
                nc.sync.dma_start(trib[:], tri_d.ap())

                nc.vector.memset(ones8[:], 1.0)
                for t16 in range(16):
                    nc.vector.tensor_copy(vA[:, t16, :, 64:65], ones8.unsqueeze(2))

                def rope(pq, dst2d, sls):
                    # pqb = bf16(pq) on ScalarE (pays the PSUM read once);
                    # then all-bf16 SBUF muls run at DVE 2x: tA = pqb*cos,
                    # tB = swap32(pqb)*sin with the sign folded into the
                    # (destination-row) sin table; r = tA + tB.
                    pqb = rt.tile([P, 512], BF16, tag="pqb")
                    nc.scalar.activation(pqb[:], pq[:], Act.Copy)
                    tA = rt.tile([P, 512], BF16, tag="tA")
                    nc.vector.tensor_mul(tA[:], pqb[:], cosb[:, sls])
                    tB = rt.tile([P, 512], BF16, tag="tB")
                    for hb in (0, 64):
                        # split the swapped sin-muls across DVE and the
                        # otherwise-idle GpSimd engine
                        nc.vector.tensor_mul(
                            tB[hb : hb + 32, :],
                            pqb[hb + 32 : hb + 64, :], sinb[hb + 32 : hb + 64, sls],
                        )
                        nc.gpsimd.tensor_mul(
                            tB[hb + 32 : hb + 64, :],
                            pqb[hb : hb + 32, :], sinb[hb : hb + 32, sls],
                        )
                    nc.vector.tensor_add(dst2d, tA[:], tB[:])

                for sl in range(4):
                    sls = ts(sl, 512)
                    if sl == 0:
                        xs = xs0
                    else:
                        xs = x1.tile([P, 8, 512], BF16, tag="xs")
                        for kc in range(8):
                            nc.sync.dma_start(xs[:, kc, :], xT3[:, kc, sls])
                    for jc in range(4):
                        pq = ps1.tile([P, 512], F32, tag="p1")
                        for kc in range(8):
                            nc.tensor.matmul(
                                pq[:], wq_s[:, kc, ts(jc, P)], xs[:, kc, :],
                                start=(kc == 0), stop=(kc == 7),
                            )
                        rope(pq, qT[:, jc, sls], sls)
                        pk = ps1.tile([P, 512], F32, tag="p1")
                        for kc in range(8):
                            nc.tensor.matmul(
                                pk[:], wk_s[:, kc, ts(jc, P)], xs[:, kc, :],
                                start=(kc == 0), stop=(kc == 7),
                            )
                        rope(pk, kT[:, jc, sls], sls)
                    for t4i in range(4):
                        pv = ps1.tile([P, 512], F32, tag="p1")
                        for kc in range(8):
                            nc.tensor.matmul(
                                pv[:], xs[:, kc, ts(t4i, P)], wv_s[:, kc, :],
                                start=(kc == 0), stop=(kc == 7),
                            )
                        nc.vector.tensor_copy(
                            vA[:, sl * 4 + t4i, :, 0:64],
                            pv.rearrange("p (h c) -> p h c", h=8),
                        )

            # ---- Phase 2: attention (query-block outer) + interleaved
            # ---- phase 3 (output projection y = outT.T @ woT) ----
            with (
                tc.tile_pool(name="ptp", bufs=4) as ptp,
                tc.tile_pool(name="rcp", bufs=4) as rcp,
                tc.tile_pool(name="rbp", bufs=4) as rbp,
                tc.tile_pool(name="ysb", bufs=3) as ysb,
                tc.tile_pool(name="drm", bufs=2, space="DRAM") as drm,
                tc.tile_pool(name="psB", bufs=2, space="PSUM") as psB,
                tc.tile_pool(name="psC", bufs=3, space="PSUM") as psC,
                tc.tile_pool(name="ps3", bufs=1, space="PSUM") as ps3,
            ):

                def attention_block(j, hc):
                    pa0 = psC.tile([65, 512], F32, tag="pa")
                    pa1 = psC.tile([65, 512], F32, tag="pa")
                    last = 4 * j + 3
                    for i in range(last + 1):
                        m = i - 4 * j
                        w0 = max(m, 0) * P
                        sc = psB.tile([P, 1024], F32, tag="sc")
                        nc.tensor.matmul(
                            sc[:, w0:512], kT[0:64, hc, ts(i, P)],
                            qT[0:64, hc, j * 512 + w0 : (j + 1) * 512],
                            start=True, stop=True,
                        )
                        nc.tensor.matmul(
                            sc[:, 512 + w0 : 1024], kT[64:P, hc, ts(i, P)],
                            qT[64:P, hc, j * 512 + w0 : (j + 1) * 512],
                            start=True, stop=True,
                        )
                        pt = ptp.tile([P, 1024], BF16, tag="pt")
                        if m < 0:
                            nc.scalar.activation(pt[:], sc[:], Act.Exp)
                        else:
                            nc.scalar.activation(
                                pt[:, w0:512], sc[:, w0:512], Act.Exp
                            )
                            nc.scalar.activation(
                                pt[:, 512 + w0 : 1024],
                                sc[:, 512 + w0 : 1024], Act.Exp,
                            )
                            nc.vector.tensor_mul(
                                pt[:, w0 : w0 + P], pt[:, w0 : w0 + P], trib[:]
                            )
                            nc.vector.tensor_mul(
                                pt[:, 512 + w0 : 512 + w0 + P],
                                pt[:, 512 + w0 : 512 + w0 + P], trib[:],
                            )
                        nc.tensor.matmul(
                            pa0[:, w0:512], vA[:, i, 2 * hc, :], pt[:, w0:512],
                            start=(i == 0), stop=(i == last),
                        )
                        nc.tensor.matmul(
                            pa1[:, w0:512], vA[:, i, 2 * hc + 1, :],
                            pt[:, 512 + w0 : 1024],
                            start=(i == 0), stop=(i == last),
                        )
                    den_d = den_tiles[j]
                    for h01, pa in ((0, pa0), (1, pa1)):
                        hb = h01 * 64
                        nc.vector.tensor_copy(
                            outT[hb : hb + 64, hc, ts(j, 512)], pa[0:64, :]
                        )
                        # cheap pa release: denominator row to SBUF, then to
                        # the j-batched DRAM staging tile (reciprocal is a
                        # multi-pass DVE composite — run it once per j on all
                        # 8 rows, not per head)
                        r = 2 * hc + h01
                        srow = rcp.tile([1, 512], F32, tag="srow")
                        nc.vector.tensor_copy(srow[:], pa[64:65, :])
                        nc.sync.dma_start(den_d[r : r + 1, :], srow[:])

                def norm_block(j):
                    # one batched reciprocal over the j's 8 denominator rows,
                    # broadcast the rows back through DRAM, multiply outT's
                    # j-block in place
                    den_sb = rcp.tile([8, 512], F32, tag="densb")
                    nc.sync.dma_start(den_sb[:], den_tiles[j][:])
                    rec = rcp.tile([8, 512], BF16, tag="rec")
                    with nc.allow_low_precision(reason="bf16 softmax normalizer"):
                        nc.vector.reciprocal(rec[:], den_sb[:])
                    rec_d = drm.tile([8, 512], BF16, tag="recd", name="recd")
                    nc.sync.dma_start(rec_d[:], rec[:])
                    for hc in range(4):
                        for h01 in range(2):
                            r = 2 * hc + h01
                            hb = h01 * 64
                            rb = rbp.tile([P, 512], BF16, tag="rb")
                            nc.sync.dma_start(
                                rb[hb : hb + 64, :],
                                rec_d[r : r + 1, :].broadcast_to((64, 512)),
                            )
                            nc.vector.tensor_mul(
                                outT[hb : hb + 64, hc, ts(j, 512)],
                                outT[hb : hb + 64, hc, ts(j, 512)],
                                rb[hb : hb + 64, :],
                            )

                def p3_group(j, sts):
                    for st in sts:
                        for half in range(2):
                            py = ps3.tile([P, 512], F32, tag="py")
                            for jc in range(4):
                                nc.tensor.matmul(
                                    py[:], outT[:, jc, ts(st, P)],
                                    wo_s[:, jc, half * 512 : (half + 1) * 512],
                                    start=(jc == 0), stop=(jc == 3),
                                )
                            yo = ysb.tile([P, 512], F32, tag="yo")
                            nc.vector.tensor_copy(yo[:], py[:])
                            nc.sync.dma_start(
                                y_ap[ts(st, P), half * 512 : (half + 1) * 512],
                                yo[:],
                            )

                den_tiles = {}
                for j in range(4):
                    den_tiles[j] = drm.tile([8, 512], F32, tag="dend", name="dend")
                    for hc in range(4):
                        attention_block(j, hc)
                        # p3 of the previous block interleaves into this
                        # block's PE stream, two query-chunks at a time,
                        # starting late enough that norm(j-1) has resolved
                        if j >= 1 and hc >= 2:
                            p3_group(j - 1, [4 * (j - 1) + 2 * (hc - 2) + k for k in range(2)])
                    norm_block(j)
                p3_group(3, [12, 13, 14, 15])

    nc.compile()
    return nc


def prep_core_inputs(x, token_ids, Wq, Wk, Wv, Wo, core):
    import ml_dtypes

    bf16 = ml_dtypes.bfloat16
    b, half = divmod(core, 2)
    rows = []
    for h in range(half * 8, half * 8 + 8):
        base = h * DH
        rows.extend(base + np.arange(0, DH, 2))
        rows.extend(base + np.arange(1, DH, 2))
    rows = np.asarray(rows)
    cols = np.arange(half * 512, half * 512 + 512)

    f32 = np.float32
    inv = THETA ** (-np.arange(0, DH, 2, dtype=np.float64) / DH)
    ang = np.asarray(token_ids, dtype=np.float64)[None, :] * inv[:, None]
    cosT = np.tile(np.cos(ang), (4, 1)).astype(bf16)
    # sign folded per SOURCE row (verifier wants both mul inputs on the same
    # partitions): odd-dim source rows carry -sin (r1 = x1 c - x2 s), even-dim
    # source rows carry +sin (r2 = x2 c + x1 s)
    sin_block = np.concatenate([np.sin(ang), -np.sin(ang)], axis=0)
    sinT = np.tile(sin_block, (2, 1)).astype(bf16)
    tri = (np.arange(P)[:, None] <= np.arange(P)[None, :]).astype(bf16)
    return {
        "xT": np.ascontiguousarray(np.asarray(x, f32)[b].T).astype(bf16),
        "wqT": np.ascontiguousarray((np.asarray(Wq, f32)[rows] * 0.125).T).astype(bf16),
        "wkT": np.ascontiguousarray(np.asarray(Wk, f32)[rows].T).astype(bf16),
        "wvT": np.ascontiguousarray(np.asarray(Wv, f32)[cols].T).astype(bf16),
        "woT": np.ascontiguousarray(np.asarray(Wo, f32)[:, cols].T).astype(bf16),
        "cosT": cosT,
        "sinT": sinT,
        "tri": tri,
    }


def get_nc():
    if "nc" not in _CACHE:
        _CACHE["nc"] = build_nc()
    return _CACHE["nc"]


def run_cores(in_maps, trace=False):
    from concourse.bass_utils import run_bass_kernel_spmd

    return run_bass_kernel_spmd(
        get_nc(), in_maps, core_ids=list(range(N_CORES)), trace=trace
    )


def kernel(x, token_ids, Wq, Wk, Wv, Wo):
    in_maps = [
        prep_core_inputs(x, token_ids, Wq, Wk, Wv, Wo, c) for c in range(N_CORES)
    ]
    res = run_cores(in_maps)
    y = np.empty((B, S, D), np.float32)
    for b in range(B):
        y[b] = res.results[2 * b]["y"] + res.results[2 * b + 1]["y"]
    return y


# revision 20
# speedup vs baseline: 1.4173x; 1.1177x over previous
"""Causal multi-head self-attention (B=4, S=2048, D=1024, H=16) on 8 TRN2
NeuronCores.

Sharding: core c = (batch b = c//2, head-half = c%2). Each core computes, for
its batch and its 8 heads: fused QKV projections (+RoPE), causal softmax
attention, and a row-sharded output projection; the host sums the two partial
y's per batch.

v2 (vs the fp32r baseline): all matmul operands are bf16 (halves DMA, removes
the fp32r narrow-moving 4x penalty, enables DVE 2x modes); q/k/v projections
share one pass over x (no phase boundary); RoPE's partition swap runs as 4
small SBUF->SBUF DMAs off the compute engines (ScalarE downcasts the PSUM
projection to bf16, DVE does 2 muls + 1 add at 2x) instead of 4 full-price
partition-sliced DVE muls; attention is query-block-outer so the output
projection of block j interleaves into block j+1's PE stream; per-head [65,512]
PSUM accumulators (+ ones column emitting softmax denominators) keep all 8
PSUM banks allocated: 2x2 scores + 3 pa + 1 output-projection.

Device layouts (per core):
  qT,kT: [128, 4, 2048] bf16; chunk hc = heads (2hc, 2hc+1); within a head's 64
         rows: [even dims (32) | odd dims (32)].
  v:     [128, 16, 8, 65] bf16 = [t % 128, t//128, head, dim+ones]; the ones
         column makes the AV matmul emit the softmax denominator as row 64.
  scores are built transposed (S.T[t, s]) so exp(S.T) feeds the AV matmul as
  the moving operand with no transposes anywhere.
"""

import numpy as np

B, S, D = 4, 2048, 1024
NUM_HEADS = 16
THETA = 10000.0
DH = 64
N_CORES = 8
P = 128

_CACHE = {}


def build_nc():
    """Build the single-core SPMD Bass program (identical on all 8 cores)."""
    import concourse.mybir as mybir
    import concourse.tile as tile
    from concourse import bacc
    from concourse.bass import ts

    F32 = mybir.dt.float32
    BF16 = mybir.dt.bfloat16
    Act = mybir.ActivationFunctionType

    nc = bacc.Bacc(trn_type="TRN2")
    xT_d = nc.dram_tensor("xT", [D, S], BF16, kind="ExternalInput")
    wqT_d = nc.dram_tensor("wqT", [D, 512], BF16, kind="ExternalInput")
    wkT_d = nc.dram_tensor("wkT", [D, 512], BF16, kind="ExternalInput")
    wvT_d = nc.dram_tensor("wvT", [D, 512], BF16, kind="ExternalInput")
    woT_d = nc.dram_tensor("woT", [512, D], BF16, kind="ExternalInput")
    cosT_d = nc.dram_tensor("cosT", [P, S], BF16, kind="ExternalInput")
    sinT_d = nc.dram_tensor("sinT", [P, S], BF16, kind="ExternalInput")
    tri_d = nc.dram_tensor("tri", [P, P], BF16, kind="ExternalInput")
    y_d = nc.dram_tensor("y", [S, D], F32, kind="ExternalOutput")

    xT3 = xT_d.ap().rearrange("(kc p) s -> p kc s", p=P)     # [128, 8, 2048]
    wq3 = wqT_d.ap().rearrange("(kc p) j -> p kc j", p=P)    # [128, 8, 512]
    wk3 = wkT_d.ap().rearrange("(kc p) j -> p kc j", p=P)
    wv3 = wvT_d.ap().rearrange("(kc p) j -> p kc j", p=P)
    wo3 = woT_d.ap().rearrange("(jc p) i -> p jc i", p=P)    # [128, 4, 1024]
    y_ap = y_d.ap()

    with tile.TileContext(nc) as tc:
        with tc.tile_pool(name="pers", bufs=1) as pers:
            qT = pers.tile([P, 4, S], BF16)
            kT = pers.tile([P, 4, S], BF16)
            vA = pers.tile([P, 16, 8, 65], BF16)
            outT = pers.tile([P, 4, S], BF16)
            wo_s = pers.tile([P, 4, D], BF16)
            trib = pers.tile([P, P], BF16)
            cosb = pers.tile([P, S], BF16)
            sinb = pers.tile([P, S], BF16)

            # ---- Phase 1: fused q/k/v projections + RoPE ----
            with (
                tc.tile_pool(name="w1", bufs=1) as w1,
                tc.tile_pool(name="x1", bufs=2) as x1,
                tc.tile_pool(name="rt", bufs=3) as rt,
                tc.tile_pool(name="ps1", bufs=4, space="PSUM") as ps1,
            ):
                wq_s = w1.tile([P, 8, 512], BF16)
                wk_s = w1.tile([P, 8, 512], BF16)
                wv_s = w1.tile([P, 8, 512], BF16)
                ones8 = w1.tile([P, 8], F32)

                # DMA priority order: first q-projection inputs, then rope
                # tables / k / v weights, then the phase-2/3 constants.
                xs0 = x1.tile([P, 8, 512], BF16, tag="xs")
                for kc in range(8):
                    nc.sync.dma_start(xs0[:, kc, :], xT3[:, kc, 0:512])
                    nc.sync.dma_start(wq_s[:, kc, :], wq3[:, kc, :])
                nc.sync.dma_start(cosb[:], cosT_d.ap())
                nc.sync.dma_start(sinb[:], sinT_d.ap())
                for kc in range(8):
                    nc.sync.dma_start(wk_s[:, kc, :], wk3[:, kc, :])
                nc.sync.dma_start(wv_s[:], wv3)
                nc.sync.dma_start(wo_s[:], wo3)
                nc.sync.dma_start(trib[:], tri_d.ap())

                nc.vector.memset(ones8[:], 1.0)
                for t16 in range(16):
                    nc.vector.tensor_copy(vA[:, t16, :, 64:65], ones8.unsqueeze(2))

                # RoPE strategy: partial-partition (channels<128) engine ops
                # cost ~3x on HW, so every DVE op here is full-channel. Per
                # (sl, matrix): ScalarE stages the 4 projection chunks into
                # pall (bf16); the 32-row partition swap runs as 4 batched
                # SBUF->SBUF DMAs over the whole staging tile; DVE then does
                # 3 full-width ops per chunk: dst = pall*cos; tB = psw*sinS
                # (sign folded per destination row); dst += tB.
                def rope_finish(pall, dstT, sls):
                    psw = rt.tile([P, 4, 512], BF16, tag="psw")
                    for hb in (0, 64):
                        nc.sync.dma_start(
                            psw[hb : hb + 32, :, :], pall[hb + 32 : hb + 64, :, :]
                        )
                        nc.sync.dma_start(
                            psw[hb + 32 : hb + 64, :, :], pall[hb : hb + 32, :, :]
                        )
                    for jc in range(4):
                        tB = rt.tile([P, 512], BF16, tag="tB")
                        nc.vector.tensor_mul(tB[:], psw[:, jc, :], sinb[:, sls])
                        nc.vector.tensor_add(
                            dstT[:, jc, sls], dstT[:, jc, sls], tB[:]
                        )

                for sl in range(4):
                    sls = ts(sl, 512)
                    if sl == 0:
                        xs = xs0
                    else:
                        xs = x1.tile([P, 8, 512], BF16, tag="xs")
                        for kc in range(8):
                            nc.sync.dma_start(xs[:, kc, :], xT3[:, kc, sls])
                    qall = rt.tile([P, 4, 512], BF16, tag="pall")
                    kall = rt.tile([P, 4, 512], BF16, tag="pall")
                    for w_s, pall, dstT in ((wq_s, qall, qT), (wk_s, kall, kT)):
                        for jc in range(4):
                            pq = ps1.tile([P, 512], F32, tag="p1")
                            for kc in range(8):
                                nc.tensor.matmul(
                                    pq[:], w_s[:, kc, ts(jc, P)], xs[:, kc, :],
                                    start=(kc == 0), stop=(kc == 7),
                                )
                            nc.scalar.activation(pall[:, jc, :], pq[:], Act.Copy)
                            nc.vector.tensor_mul(
                                dstT[:, jc, sls], pall[:, jc, :], cosb[:, sls]
                            )
                    for t4i in range(4):
                        pv = ps1.tile([P, 512], F32, tag="p1")
                        for kc in range(8):
                            nc.tensor.matmul(
                                pv[:], xs[:, kc, ts(t4i, P)], wv_s[:, kc, :],
                                start=(kc == 0), stop=(kc == 7),
                            )
                        nc.vector.tensor_copy(
                            vA[:, sl * 4 + t4i, :, 0:64],
                            pv.rearrange("p (h c) -> p h c", h=8),
                        )
                    rope_finish(qall, qT, sls)
                    rope_finish(kall, kT, sls)

            # ---- Phase 2: attention (query-block outer) + interleaved
            # ---- phase 3 (output projection y = outT.T @ woT) ----
            with (
                tc.tile_pool(name="ptp", bufs=4) as ptp,
                tc.tile_pool(name="rcp", bufs=4) as rcp,
                tc.tile_pool(name="rbp", bufs=4) as rbp,
                tc.tile_pool(name="ysb", bufs=3) as ysb,
                tc.tile_pool(name="drm", bufs=2, space="DRAM") as drm,
                tc.tile_pool(name="psB", bufs=2, space="PSUM") as psB,
                tc.tile_pool(name="psC", bufs=3, space="PSUM") as psC,
                tc.tile_pool(name="ps3", bufs=1, space="PSUM") as ps3,
            ):

                def attention_block(j, hc):
                    pa0 = psC.tile([65, 512], F32, tag="pa")
                    pa1 = psC.tile([65, 512], F32, tag="pa")
                    last = 4 * j + 3
                    for i in range(last + 1):
                        m = i - 4 * j
                        w0 = max(m, 0) * P
                        sc = psB.tile([P, 1024], F32, tag="sc")
                        nc.tensor.matmul(
                            sc[:, w0:512], kT[0:64, hc, ts(i, P)],
                            qT[0:64, hc, j * 512 + w0 : (j + 1) * 512],
                            start=True, stop=True,
                        )
                        nc.tensor.matmul(
                            sc[:, 512 + w0 : 1024], kT[64:P, hc, ts(i, P)],
                            qT[64:P, hc, j * 512 + w0 : (j + 1) * 512],
                            start=True, stop=True,
                        )
                        pt = ptp.tile([P, 1024], BF16, tag="pt")
                        if m < 0:
                            nc.scalar.activation(pt[:], sc[:], Act.Exp)
                        else:
                            nc.scalar.activation(
                                pt[:, w0:512], sc[:, w0:512], Act.Exp
                            )
                            nc.scalar.activation(
                                pt[:, 512 + w0 : 1024],
                                sc[:, 512 + w0 : 1024], Act.Exp,
                            )
                            nc.vector.tensor_mul(
                                pt[:, w0 : w0 + P], pt[:, w0 : w0 + P], trib[:]
                            )
                            nc.vector.tensor_mul(
                                pt[:, 512 + w0 : 512 + w0 + P],
                                pt[:, 512 + w0 : 512 + w0 + P], trib[:],
                            )
                        nc.tensor.matmul(
                            pa0[:, w0:512], vA[:, i, 2 * hc, :], pt[:, w0:512],
                            start=(i == 0), stop=(i == last),
                        )
                        nc.tensor.matmul(
                            pa1[:, w0:512], vA[:, i, 2 * hc + 1, :],
                            pt[:, 512 + w0 : 1024],
                            start=(i == 0), stop=(i == last),
                        )
                    den_d = den_tiles[j]
                    for h01, pa in ((0, pa0), (1, pa1)):
                        hb = h01 * 64
                        nc.vector.tensor_copy(
                            outT[hb : hb + 64, hc, ts(j, 512)], pa[0:64, :]
                        )
                        # cheap pa release: denominator row to SBUF, then to
                        # the j-batched DRAM staging tile (reciprocal is a
                        # multi-pass DVE composite — run it once per j on all
                        # 8 rows, not per head)
                        r = 2 * hc + h01
                        srow = rcp.tile([1, 512], F32, tag="srow")
                        nc.vector.tensor_copy(srow[:], pa[64:65, :])
                        nc.sync.dma_start(den_d[r : r + 1, :], srow[:])

                def norm_block(j):
                    # one batched reciprocal over the j's 8 denominator rows,
                    # broadcast the rows back through DRAM, multiply outT's
                    # j-block in place
                    den_sb = rcp.tile([8, 512], F32, tag="densb")
                    nc.sync.dma_start(den_sb[:], den_tiles[j][:])
                    rec = rcp.tile([8, 512], BF16, tag="rec")
                    with nc.allow_low_precision(reason="bf16 softmax normalizer"):
                        nc.vector.reciprocal(rec[:], den_sb[:])
                    rec_d = drm.tile([8, 512], BF16, tag="recd", name="recd")
                    nc.sync.dma_start(rec_d[:], rec[:])
                    for hc in range(4):
                        for h01 in range(2):
                            r = 2 * hc + h01
                            hb = h01 * 64
                            rb = rbp.tile([P, 512], BF16, tag="rb")
                            nc.sync.dma_start(
                                rb[hb : hb + 64, :],
                                rec_d[r : r + 1, :].broadcast_to((64, 512)),
                            )
                            nc.vector.tensor_mul(
                                outT[hb : hb + 64, hc, ts(j, 512)],
                                outT[hb : hb + 64, hc, ts(j, 512)],
                                rb[hb : hb + 64, :],
                            )

                def p3_group(j, sts):
                    for st in sts:
                        for half in range(2):
                            py = ps3.tile([P, 512], F32, tag="py")
                            for jc in range(4):
                                nc.tensor.matmul(
                                    py[:], outT[:, jc, ts(st, P)],
                                    wo_s[:, jc, half * 512 : (half + 1) * 512],
                                    start=(jc == 0), stop=(jc == 3),
                                )
                            yo = ysb.tile([P, 512], F32, tag="yo")
                            nc.vector.tensor_copy(yo[:], py[:])
                            nc.sync.dma_start(
                                y_ap[ts(st, P), half * 512 : (half + 1) * 512],
                                yo[:],
                            )

                # largest query block first: its output projection interleaves
                # into the next block's PE stream, and the exposed tail is only
                # the smallest block's norm + projection
                den_tiles = {}
                js = [3, 2, 1, 0]
                for idx, j in enumerate(js):
                    den_tiles[j] = drm.tile([8, 512], F32, tag="dend", name="dend")
                    for hc in range(4):
                        attention_block(j, hc)
                        # p3 of the previously finished block interleaves into
                        # this block's PE stream, two query-chunks at a time,
                        # starting late enough that its norm has resolved
                        if idx >= 1 and hc >= 2:
                            pj = js[idx - 1]
                            p3_group(pj, [4 * pj + 2 * (hc - 2) + k for k in range(2)])
                    norm_block(j)
                p3_group(0, [0, 1, 2, 3])

    nc.compile()
    return nc


def prep_core_inputs(x, token_ids, Wq, Wk, Wv, Wo, core):
    import ml_dtypes

    bf16 = ml_dtypes.bfloat16
    b, half = divmod(core, 2)
    rows = []
    for h in range(half * 8, half * 8 + 8):
        base = h * DH
        rows.extend(base + np.arange(0, DH, 2))
        rows.extend(base + np.arange(1, DH, 2))
    rows = np.asarray(rows)
    cols = np.arange(half * 512, half * 512 + 512)

    f32 = np.float32
    inv = THETA ** (-np.arange(0, DH, 2, dtype=np.float64) / DH)
    ang = np.asarray(token_ids, dtype=np.float64)[None, :] * inv[:, None]
    cosT = np.tile(np.cos(ang), (4, 1)).astype(bf16)
    # sign folded per DESTINATION row (the swap happens in the DMA, so the
    # mul is row-aligned): even-dim rows get -sin (r1 = x1 c - x2 s), odd-dim
    # rows get +sin (r2 = x2 c + x1 s)
    sin_block = np.concatenate([-np.sin(ang), np.sin(ang)], axis=0)
    sinT = np.tile(sin_block, (2, 1)).astype(bf16)
    tri = (np.arange(P)[:, None] <= np.arange(P)[None, :]).astype(bf16)
    return {
        "xT": np.ascontiguousarray(np.asarray(x, f32)[b].T).astype(bf16),
        "wqT": np.ascontiguousarray((np.asarray(Wq, f32)[rows] * 0.125).T).astype(bf16),
        "wkT": np.ascontiguousarray(np.asarray(Wk, f32)[rows].T).astype(bf16),
        "wvT": np.ascontiguousarray(np.asarray(Wv, f32)[cols].T).astype(bf16),
        "woT": np.ascontiguousarray(np.asarray(Wo, f32)[:, cols].T).astype(bf16),
        "cosT": cosT,
        "sinT": sinT,
        "tri": tri,
    }


def get_nc():
    if "nc" not in _CACHE:
        _CACHE["nc"] = build_nc()
    return _CACHE["nc"]


def run_cores(in_maps, trace=False):
    from concourse.bass_utils import run_bass_kernel_spmd

    return run_bass_kernel_spmd(
        get_nc(), in_maps, core_ids=list(range(N_CORES)), trace=trace
    )


def kernel(x, token_ids, Wq, Wk, Wv, Wo):
    in_maps = [
        prep_core_inputs(x, token_ids, Wq, Wk, Wv, Wo, c) for c in range(N_CORES)
    ]
    res = run_cores(in_maps)
    y = np.empty((B, S, D), np.float32)
    for b in range(B):
        y[b] = res.results[2 * b]["y"] + res.results[2 * b + 1]["y"]
    return y
